# revision 21
# baseline (speedup 1.0000x reference)
"""Causal self-attention on 8 Trainium2 NeuronCores.

Full inputs in, full output out. Sharding: core c -> (batch b = c//2,
head-group hg = c%2 covering 8 of 16 heads). Each core computes QKV
projections for its head slice, causal flash-attention in a transposed
layout (S^T = keys x queries so PV needs no transposes; softmax
denominators ride the PV matmul as a 65th V column), and a partial
output projection over its 512 feature columns. The host sums the two
partials per batch and adds the bias.

v2: single software-pipelined emission — QKV for token-block tb+1 and
the deferred output projection of tb-1 are woven between attention
iterations of tb, so the tensor engine fills the exp-wait gaps and the
phases fully overlap (the scheduler gap-fills by readiness; one pool
scope avoids the SBUF-reuse barrier that serialized the phases in v1).
The softmax denominator broadcast moved off the tensor engine to a
GpSimd partition_broadcast. Matmul operands are float16 (full PE rate,
fp32 PSUM accumulation). The two heads of a feature block issue S^T
matmuls on disjoint PE row groups (partitions 0-63 / 64-127) so they
run concurrently, and diagonal blocks only compute the causally-live
trapezoid of columns.
"""
import sys

if "/opt/trn_rl_repo" not in sys.path:
    sys.path.insert(0, "/opt/trn_rl_repo")

import numpy as np

import concourse.bass as bass
import concourse.tile as tile
from concourse import bacc, library_config, mybir
from concourse.bass_utils import run_bass_kernel_spmd

F32 = mybir.dt.float32
F16 = mybir.dt.float16
AF = mybir.ActivationFunctionType

B, T, C = 4, 2048, 1024
H, D = 16, 64
N_CORES = 8
HPC = 8            # heads per core
FPC = HPC * D      # feats per core = 512
QB = 512           # query block
NQB = T // QB      # 4
NKK = T // 128     # 16 key chunks
NCC = C // 128     # 8 contraction chunks
NFB = FPC // 128   # 4 feature blocks (head pairs)

_cached = {}


def _build_program():
    nc = bacc.Bacc("TRN2", target_bir_lowering=False, debug=False,
                   num_devices=N_CORES)

    xT_d = nc.dram_tensor("xT", [C, T], F16, kind="ExternalInput").ap()
    wqT_d = nc.dram_tensor("wqT", [C, FPC], F16, kind="ExternalInput").ap()
    wkT_d = nc.dram_tensor("wkT", [C, FPC], F16, kind="ExternalInput").ap()
    wvT_d = nc.dram_tensor("wvT", [C, FPC], F16, kind="ExternalInput").ap()
    wpT_d = nc.dram_tensor("wpT", [FPC, C], F16, kind="ExternalInput").ap()
    ident_d = nc.dram_tensor("ident", [128, 128], F16, kind="ExternalInput").ap()
    tri_d = nc.dram_tensor("tri", [128, 128], F16, kind="ExternalInput").ap()
    out_d = nc.dram_tensor("out", [T, C], F32, kind="ExternalOutput").ap()

    wq_r = wqT_d.rearrange("(c p) f -> p c f", p=128)
    wk_r = wkT_d.rearrange("(c p) f -> p c f", p=128)
    wv_r = wvT_d.rearrange("(c p) f -> p c f", p=128)
    xT_r = xT_d.rearrange("(c p) t -> p c t", p=128)

    with tile.TileContext(nc) as tc:
        with tc.tile_pool(name="persist", bufs=1) as persist, \
             tc.tile_pool(name="xt", bufs=2) as xtp, \
             tc.tile_pool(name="mm", bufs=2, space="PSUM") as mmp, \
             tc.tile_pool(name="st", bufs=2, space="PSUM") as stp, \
             tc.tile_pool(name="pv", bufs=1, space="PSUM") as pvp, \
             tc.tile_pool(name="pt", bufs=4) as ptp, \
             tc.tile_pool(name="ye", bufs=3) as yep, \
             tc.tile_pool(name="rc1", bufs=3) as rc1p, \
             tc.tile_pool(name="rcb", bufs=3) as rcbp, \
             tc.tile_pool(name="osb", bufs=3) as osbp:
            qt_sb = persist.tile([128, NFB, T], F16, tag="qt")
            kt_sb = persist.tile([128, NFB, T], F16, tag="kt")
            v_sb = persist.tile([128, NKK, HPC, D + 1], F16, tag="v")
            yt_sb = persist.tile([128, NQB, NFB, QB], F16, tag="yt")
            ident = persist.tile([128, 128], F16, tag="ident")
            tri = persist.tile([128, 128], F16, tag="tri")
            wq_sb = persist.tile([128, NCC, FPC], F16, tag="wq")
            wk_sb = persist.tile([128, NCC, FPC], F16, tag="wk")
            wv_sb = persist.tile([128, NCC, FPC], F16, tag="wv")
            wp_sb = persist.tile([128, NFB, C], F16, tag="wp")

            # gpsimd: the attn ucode library provides partition_broadcast
            nc.gpsimd.memset(v_sb[:, :, :, D:D + 1], 1.0)
            nc.gpsimd.load_library(library_config.attn)

            nc.sync.dma_start(ident, ident_d)
            nc.sync.dma_start(tri, tri_d)

            xt_tiles = [None] * NQB
            xt_tiles[0] = xtp.tile([128, NCC, QB], F16, tag="xt", name="xt")
            for cc in range(NCC):
                nc.sync.dma_start(xt_tiles[0][:, cc, :], xT_r[:, cc, 0:QB])
            for cc in range(NCC):
                nc.sync.dma_start(wq_sb[:, cc, :], wq_r[:, cc, :])
            for cc in range(NCC):
                nc.sync.dma_start(wk_sb[:, cc, :], wk_r[:, cc, :])
            for cc in range(NCC):
                nc.sync.dma_start(wv_sb[:, cc, :], wv_r[:, cc, :])
            nc.sync.dma_start(
                wp_sb, wpT_d.rearrange("(c p) f -> p c f", p=128))

            # warm the PE clock gate while input DMAs stream in (the st
            # pool is unused until attention starts)
            warm = stp.tile([128, 2, QB], F32, tag="st", name="warm")
            for _ in range(80):
                nc.tensor.matmul(warm[:, 0, 0:128], ident, ident,
                                 start=True, stop=True,
                                 skip_group_check=True)

            # ---------------- emission helpers ----------------
            def emit_xt_dma(tb):
                xt = xtp.tile([128, NCC, QB], F16, tag="xt", name="xt")
                xt_tiles[tb] = xt
                for cc in range(NCC):
                    nc.sync.dma_start(xt[:, cc, :],
                                      xT_r[:, cc, tb * QB:(tb + 1) * QB])

            def emit_qk_group(tb, fb, w_sb, dst):
                xt = xt_tiles[tb]
                ps = mmp.tile([128, 512], F32, tag="mm", name="mm")
                for cc in range(NCC):
                    nc.tensor.matmul(
                        ps, w_sb[:, cc, fb * 128:(fb + 1) * 128],
                        xt[:, cc, :],
                        start=(cc == 0), stop=(cc == NCC - 1))
                nc.vector.tensor_copy(
                    dst[:, fb, tb * QB:(tb + 1) * QB], ps)

            def emit_v_group(tb, tt):
                xt = xt_tiles[tb]
                ps = mmp.tile([128, 512], F32, tag="mm", name="mm")
                for cc in range(NCC):
                    nc.tensor.matmul(
                        ps, xt[:, cc, tt * 128:(tt + 1) * 128],
                        wv_sb[:, cc, :],
                        start=(cc == 0), stop=(cc == NCC - 1))
                nc.vector.tensor_copy(
                    v_sb[:, tb * 4 + tt, :, 0:D],
                    ps.rearrange("p (h d) -> p h d", h=HPC))

            def qkv_fills(tb):
                items = [lambda tb=tb: emit_xt_dma(tb)]
                for fb in range(NFB):
                    items.append(lambda tb=tb, fb=fb: emit_qk_group(
                        tb, fb, wq_sb, qt_sb))
                    items.append(lambda tb=tb, fb=fb: emit_qk_group(
                        tb, fb, wk_sb, kt_sb))
                for tt in range(4):
                    items.append(lambda tb=tb, tt=tt: emit_v_group(tb, tt))
                return items

            def emit_proj_group(tb, tt):
                osb = osbp.tile([128, C], F32, tag="osb", name="osb")
                for ofc in range(2):
                    prj = mmp.tile([128, 512], F32, tag="mm", name="mm")
                    for cc in range(NFB):
                        nc.tensor.matmul(
                            prj,
                            yt_sb[:, tb, cc, tt * 128:(tt + 1) * 128],
                            wp_sb[:, cc, ofc * 512:(ofc + 1) * 512],
                            start=(cc == 0), stop=(cc == NFB - 1))
                    nc.vector.tensor_copy(
                        osb[:, ofc * 512:(ofc + 1) * 512], prj)
                nc.sync.dma_start(
                    out_d[tb * QB + tt * 128:tb * QB + (tt + 1) * 128, :],
                    osb)

            def proj_fills(tb):
                return [lambda tb=tb, tt=tt: emit_proj_group(tb, tt)
                        for tt in range(4)]

            # ---------------- main pipelined emission ----------------
            for fb in range(NFB):
                emit_qk_group(0, fb, wq_sb, qt_sb)
                emit_qk_group(0, fb, wk_sb, kt_sb)
            for tt in range(4):
                emit_v_group(0, tt)

            SEQUENTIAL = False
            if SEQUENTIAL:
                for tb in range(1, NQB):
                    emit_xt_dma(tb)
                    for fb in range(NFB):
                        emit_qk_group(tb, fb, wq_sb, qt_sb)
                        emit_qk_group(tb, fb, wk_sb, kt_sb)
                    for tt in range(4):
                        emit_v_group(tb, tt)

            # Two-speed fill queue: QKV(tb+1) must drain inside
            # attention(tb); proj(tb) may spread over everything after
            # attention(tb), keeping the tensor engine fed in the late
            # (scalar-bound) stretch of attention(3).
            lazy = []
            lazy_popped = 0
            lazy_acc = 0.0
            iters_of = [NFB * (4 * t + 4) for t in range(NQB)]
            for tb in range(NQB):
                fills = []
                if not SEQUENTIAL:
                    if tb < NQB - 1:
                        fills += qkv_fills(tb + 1)
                nkk = 4 * tb + 4
                total_iters = NFB * nkk
                # hold two groups back so the last-fb tail chain of
                # attention(3) has tensor work to hide behind
                lazy_rate = max(0, len(lazy) - lazy_popped - 2) \
                    / sum(iters_of[tb:])
                lazy_acc = float(lazy_popped)
                it = 0
                popped = 0

                for fb in range(NFB):
                    pv = [pvp.tile([65, QB], F32, tag=f"pv{h2}",
                                   name=f"pv{h2}")
                          for h2 in range(2)]

                    def emit_pv(kk, ptile, nkk=nkk, tb=tb, fb=fb, pv=pv):
                        dl = kk - 4 * tb
                        j0 = 128 * dl if dl >= 0 else 0
                        for h2 in range(2):
                            h = 2 * fb + h2
                            nc.tensor.matmul(
                                pv[h2][:, j0:QB], v_sb[:, kk, h, :],
                                ptile[:, h2, j0:QB],
                                start=(kk == 0), stop=(kk == nkk - 1),
                                skip_group_check=True)

                    pending = None  # (kk, ptile) whose PV is not yet emitted
                    for kk in range(nkk):
                        dl = kk - 4 * tb
                        j0 = 128 * dl if dl >= 0 else 0
                        st = stp.tile([128, 2, QB], F32, tag="st", name="st")
                        for h2 in range(2):
                            p0, p1 = 64 * h2, 64 * h2 + 64
                            nc.tensor.matmul(
                                st[:, h2, j0:QB],
                                kt_sb[p0:p1, fb, kk * 128:(kk + 1) * 128],
                                qt_sb[p0:p1, fb, tb * QB + j0:(tb + 1) * QB],
                                start=True, stop=True,
                                skip_group_check=True)
                        ptile = ptp.tile([128, 2, QB], F16, tag="pt",
                                         name="ptile")
                        nc.scalar.activation(
                            ptile[:, :, j0:QB], st[:, :, j0:QB], AF.Exp)
                        if dl >= 0:
                            # zero the causally-dead triangle of the
                            # diagonal band (both heads in one strided op;
                            # the 0-step middle dim re-reads the mask tile)
                            band = ptile[:, :, j0:j0 + 128]
                            nc.vector.tensor_mul(
                                band, band,
                                bass.AP(tri.tensor, tri.offset,
                                        [tri.ap[0], [0, 2], tri.ap[1]]))
                        # PV for kk-1 goes out now: a full iteration of
                        # tensor work sits between exp(kk-1) and its PV,
                        # so exp jitter never stalls the PE queue
                        if pending is not None:
                            emit_pv(*pending)
                        pending = (kk, ptile)
                        it += 1
                        want = len(fills) * it // total_iters
                        while popped < want:
                            fills[popped]()
                            popped += 1
                        lazy_acc += lazy_rate
                        while lazy_popped < min(int(lazy_acc), len(lazy)):
                            lazy[lazy_popped]()
                            lazy_popped += 1
                    emit_pv(*pending)

                    for h2 in range(2):
                        p0, p1 = 64 * h2, 64 * h2 + 64
                        # evacuate the unnormalized y and the denominator
                        # fast so the pv bank frees for fb+1
                        ye = yep.tile([64, QB], F16, tag="ye", name="ye")
                        nc.vector.tensor_copy(ye, pv[h2][0:D, :])
                        dcp = rc1p.tile([1, QB], F32, tag="rc1", name="dcp")
                        nc.vector.tensor_copy(dcp, pv[h2][D:D + 1, :])
                        rcp1 = rc1p.tile([1, QB], F32, tag="rc1b",
                                         name="rcp1")
                        nc.vector.reciprocal_approx_fast(
                            out=rcp1, in_=dcp)
                        rcpb = rcbp.tile([64, QB], F32, tag="rcb",
                                         name="rcb")
                        nc.gpsimd.partition_broadcast(rcpb, rcp1)
                        nc.vector.tensor_mul(yt_sb[p0:p1, tb, fb, :],
                                             ye, rcpb)
                while popped < len(fills):
                    fills[popped]()
                    popped += 1
                if not SEQUENTIAL and tb < NQB - 1:
                    lazy += proj_fills(tb)

            while lazy_popped < len(lazy):
                lazy[lazy_popped]()
                lazy_popped += 1
            if SEQUENTIAL:
                for tb in range(NQB - 1):
                    for tt in range(4):
                        emit_proj_group(tb, tt)
            for tt in range(4):
                emit_proj_group(NQB - 1, tt)

    nc.compile()
    return nc


def _host_inputs(x, Wk, Wq, Wv, Wp):
    """Build the 8 per-core input maps (host-side slicing/transposes)."""
    ident_np = np.eye(128, dtype=np.float16)
    p = np.arange(128)[:, None]
    jj = np.arange(128)[None, :]
    tri_np = np.where(jj < p, 0.0, 1.0).astype(np.float16)

    in_maps = []
    for c in range(N_CORES):
        b, hg = c // 2, c % 2
        fs = slice(hg * FPC, (hg + 1) * FPC)
        in_maps.append({
            "xT": np.ascontiguousarray(x[b].T).astype(np.float16),
            "wqT": np.ascontiguousarray((Wq[fs, :] * 0.125).T).astype(np.float16),
            "wkT": np.ascontiguousarray(Wk[fs, :].T).astype(np.float16),
            "wvT": np.ascontiguousarray(Wv[fs, :].T).astype(np.float16),
            "wpT": np.ascontiguousarray(Wp[:, fs].T).astype(np.float16),
            "ident": ident_np,
            "tri": tri_np,
        })
    return in_maps


def kernel(x, Wk, Wq, Wv, Wp, bp, _trace=False):
    x = np.asarray(x, dtype=np.float32)
    Wk = np.asarray(Wk, dtype=np.float32)
    Wq = np.asarray(Wq, dtype=np.float32)
    Wv = np.asarray(Wv, dtype=np.float32)
    Wp = np.asarray(Wp, dtype=np.float32)
    bp = np.asarray(bp, dtype=np.float32)

    if "nc" not in _cached:
        _cached["nc"] = _build_program()
    nc = _cached["nc"]

    in_maps = _host_inputs(x, Wk, Wq, Wv, Wp)
    res = run_bass_kernel_spmd(nc, in_maps, core_ids=list(range(N_CORES)),
                               trace=_trace)
    _cached["last_result"] = res

    out = np.empty((B, T, C), dtype=np.float32)
    for b in range(B):
        out[b] = (res.results[2 * b]["out"].astype(np.float32)
                  + res.results[2 * b + 1]["out"]
                  + bp[None, :])
    return out


# revision 22
# speedup vs baseline: 1.0128x; 1.0128x over previous
"""Causal self-attention on 8 Trainium2 NeuronCores.

Full inputs in, full output out. Sharding: core c -> (batch b = c//2,
head-group hg = c%2 covering 8 of 16 heads). Each core computes QKV
projections for its head slice, causal flash-attention in a transposed
layout (S^T = keys x queries so PV needs no transposes; softmax
denominators ride the PV matmul as a 65th V column), and a partial
output projection over its 512 feature columns. The host sums the two
partials per batch and adds the bias.

v2: single software-pipelined emission — QKV for token-block tb+1 and
the deferred output projection of tb-1 are woven between attention
iterations of tb, so the tensor engine fills the exp-wait gaps and the
phases fully overlap (the scheduler gap-fills by readiness; one pool
scope avoids the SBUF-reuse barrier that serialized the phases in v1).
The softmax denominator broadcast moved off the tensor engine to a
GpSimd partition_broadcast. Matmul operands are float16 (full PE rate,
fp32 PSUM accumulation). The two heads of a feature block issue S^T
matmuls on disjoint PE row groups (partitions 0-63 / 64-127) so they
run concurrently, and diagonal blocks only compute the causally-live
trapezoid of columns.
"""
import sys

if "/opt/trn_rl_repo" not in sys.path:
    sys.path.insert(0, "/opt/trn_rl_repo")

import numpy as np

import concourse.bass as bass
import concourse.tile as tile
from concourse import bacc, library_config, mybir
from concourse.bass_utils import run_bass_kernel_spmd

F32 = mybir.dt.float32
F16 = mybir.dt.float16
AF = mybir.ActivationFunctionType

B, T, C = 4, 2048, 1024
H, D = 16, 64
N_CORES = 8
HPC = 8            # heads per core
FPC = HPC * D      # feats per core = 512
QB = 512           # query block
NQB = T // QB      # 4
NKK = T // 128     # 16 key chunks
NCC = C // 128     # 8 contraction chunks
NFB = FPC // 128   # 4 feature blocks (head pairs)

_cached = {}


def _build_program():
    nc = bacc.Bacc("TRN2", target_bir_lowering=False, debug=False,
                   num_devices=N_CORES)

    xT_d = nc.dram_tensor("xT", [C, T], F16, kind="ExternalInput").ap()
    wqT_d = nc.dram_tensor("wqT", [C, FPC], F16, kind="ExternalInput").ap()
    wkT_d = nc.dram_tensor("wkT", [C, FPC], F16, kind="ExternalInput").ap()
    wvT_d = nc.dram_tensor("wvT", [C, FPC], F16, kind="ExternalInput").ap()
    wpT_d = nc.dram_tensor("wpT", [FPC, C], F16, kind="ExternalInput").ap()
    ident_d = nc.dram_tensor("ident", [128, 128], F16, kind="ExternalInput").ap()
    tri_d = nc.dram_tensor("tri", [128, 128], F16, kind="ExternalInput").ap()
    out_d = nc.dram_tensor("out", [T, C], F32, kind="ExternalOutput").ap()

    wq_r = wqT_d.rearrange("(c p) f -> p c f", p=128)
    wk_r = wkT_d.rearrange("(c p) f -> p c f", p=128)
    wv_r = wvT_d.rearrange("(c p) f -> p c f", p=128)
    xT_r = xT_d.rearrange("(c p) t -> p c t", p=128)

    with tile.TileContext(nc) as tc:
        with tc.tile_pool(name="persist", bufs=1) as persist, \
             tc.tile_pool(name="xt", bufs=3) as xtp, \
             tc.tile_pool(name="mm", bufs=2, space="PSUM") as mmp, \
             tc.tile_pool(name="st", bufs=2, space="PSUM") as stp, \
             tc.tile_pool(name="pv", bufs=1, space="PSUM") as pvp, \
             tc.tile_pool(name="pt", bufs=6) as ptp, \
             tc.tile_pool(name="ye", bufs=4) as yep, \
             tc.tile_pool(name="rc1", bufs=3) as rc1p, \
             tc.tile_pool(name="rcb", bufs=3) as rcbp, \
             tc.tile_pool(name="osb", bufs=3) as osbp:
            qt_sb = persist.tile([128, NFB, T], F16, tag="qt")
            kt_sb = persist.tile([128, NFB, T], F16, tag="kt")
            v_sb = persist.tile([128, NKK, HPC, D + 1], F16, tag="v")
            yt_sb = persist.tile([128, NQB, NFB, QB], F16, tag="yt")
            ident = persist.tile([128, 128], F16, tag="ident")
            tri = persist.tile([128, 128], F16, tag="tri")
            wq_sb = persist.tile([128, NCC, FPC], F16, tag="wq")
            wk_sb = persist.tile([128, NCC, FPC], F16, tag="wk")
            wv_sb = persist.tile([128, NCC, FPC], F16, tag="wv")
            wp_sb = persist.tile([128, NFB, C], F16, tag="wp")

            # gpsimd: the attn ucode library provides partition_broadcast
            nc.gpsimd.memset(v_sb[:, :, :, D:D + 1], 1.0)
            nc.gpsimd.load_library(library_config.attn)

            nc.sync.dma_start(ident, ident_d)
            nc.sync.dma_start(tri, tri_d)

            xt_tiles = [None] * NQB
            xt_tiles[0] = xtp.tile([128, NCC, QB], F16, tag="xt", name="xt")
            for cc in range(NCC):
                nc.sync.dma_start(xt_tiles[0][:, cc, :], xT_r[:, cc, 0:QB])
            for cc in range(NCC):
                nc.sync.dma_start(wq_sb[:, cc, :], wq_r[:, cc, :])
            for cc in range(NCC):
                nc.sync.dma_start(wk_sb[:, cc, :], wk_r[:, cc, :])
            for cc in range(NCC):
                nc.sync.dma_start(wv_sb[:, cc, :], wv_r[:, cc, :])
            nc.sync.dma_start(
                wp_sb, wpT_d.rearrange("(c p) f -> p c f", p=128))

            # warm the PE clock gate while input DMAs stream in (the st
            # pool is unused until attention starts)
            warm = stp.tile([128, 2, QB], F32, tag="st", name="warm")
            for _ in range(120):
                nc.tensor.matmul(warm[:, 0, 0:128], ident, ident,
                                 start=True, stop=True,
                                 skip_group_check=True)

            # ---------------- emission helpers ----------------
            def emit_xt_dma(tb):
                xt = xtp.tile([128, NCC, QB], F16, tag="xt", name="xt")
                xt_tiles[tb] = xt
                for cc in range(NCC):
                    nc.sync.dma_start(xt[:, cc, :],
                                      xT_r[:, cc, tb * QB:(tb + 1) * QB])

            def emit_qk_group(tb, fb, w_sb, dst):
                xt = xt_tiles[tb]
                ps = mmp.tile([128, 512], F32, tag="mm", name="mm")
                for cc in range(NCC):
                    nc.tensor.matmul(
                        ps, w_sb[:, cc, fb * 128:(fb + 1) * 128],
                        xt[:, cc, :],
                        start=(cc == 0), stop=(cc == NCC - 1))
                nc.vector.tensor_copy(
                    dst[:, fb, tb * QB:(tb + 1) * QB], ps)

            def emit_v_group(tb, tt):
                xt = xt_tiles[tb]
                ps = mmp.tile([128, 512], F32, tag="mm", name="mm")
                for cc in range(NCC):
                    nc.tensor.matmul(
                        ps, xt[:, cc, tt * 128:(tt + 1) * 128],
                        wv_sb[:, cc, :],
                        start=(cc == 0), stop=(cc == NCC - 1))
                nc.vector.tensor_copy(
                    v_sb[:, tb * 4 + tt, :, 0:D],
                    ps.rearrange("p (h d) -> p h d", h=HPC))

            def qkv_fills(tb):
                items = [lambda tb=tb: emit_xt_dma(tb)]
                for fb in range(NFB):
                    items.append(lambda tb=tb, fb=fb: emit_qk_group(
                        tb, fb, wq_sb, qt_sb))
                    items.append(lambda tb=tb, fb=fb: emit_qk_group(
                        tb, fb, wk_sb, kt_sb))
                for tt in range(4):
                    items.append(lambda tb=tb, tt=tt: emit_v_group(tb, tt))
                return items

            def emit_proj_group(tb, tt):
                osb = osbp.tile([128, C], F32, tag="osb", name="osb")
                for ofc in range(2):
                    prj = mmp.tile([128, 512], F32, tag="mm", name="mm")
                    for cc in range(NFB):
                        nc.tensor.matmul(
                            prj,
                            yt_sb[:, tb, cc, tt * 128:(tt + 1) * 128],
                            wp_sb[:, cc, ofc * 512:(ofc + 1) * 512],
                            start=(cc == 0), stop=(cc == NFB - 1))
                    nc.vector.tensor_copy(
                        osb[:, ofc * 512:(ofc + 1) * 512], prj)
                nc.sync.dma_start(
                    out_d[tb * QB + tt * 128:tb * QB + (tt + 1) * 128, :],
                    osb)

            def proj_fills(tb):
                return [lambda tb=tb, tt=tt: emit_proj_group(tb, tt)
                        for tt in range(4)]

            # ---------------- main pipelined emission ----------------
            for fb in range(NFB):
                emit_qk_group(0, fb, wq_sb, qt_sb)
                emit_qk_group(0, fb, wk_sb, kt_sb)
            for tt in range(4):
                emit_v_group(0, tt)

            SEQUENTIAL = False
            if SEQUENTIAL:
                for tb in range(1, NQB):
                    emit_xt_dma(tb)
                    for fb in range(NFB):
                        emit_qk_group(tb, fb, wq_sb, qt_sb)
                        emit_qk_group(tb, fb, wk_sb, kt_sb)
                    for tt in range(4):
                        emit_v_group(tb, tt)

            # Two-speed fill queue: QKV(tb+1) must drain inside
            # attention(tb); proj(tb) may spread over everything after
            # attention(tb), keeping the tensor engine fed in the late
            # (scalar-bound) stretch of attention(3).
            lazy = []
            lazy_popped = 0
            lazy_acc = 0.0
            iters_of = [NFB * (4 * t + 4) for t in range(NQB)]
            for tb in range(NQB):
                fills = []
                if not SEQUENTIAL:
                    if tb < NQB - 1:
                        fills += qkv_fills(tb + 1)
                nkk = 4 * tb + 4
                total_iters = NFB * nkk
                # hold two groups back so the last-fb tail chain of
                # attention(3) has tensor work to hide behind
                lazy_rate = max(0, len(lazy) - lazy_popped - 2) \
                    / sum(iters_of[tb:])
                lazy_acc = float(lazy_popped)
                it = 0
                popped = 0

                for fb in range(NFB):
                    pv = [pvp.tile([65, QB], F32, tag=f"pv{h2}",
                                   name=f"pv{h2}")
                          for h2 in range(2)]

                    def emit_pv(kk, ptile, nkk=nkk, tb=tb, fb=fb, pv=pv):
                        dl = kk - 4 * tb
                        j0 = 128 * dl if dl >= 0 else 0
                        for h2 in range(2):
                            h = 2 * fb + h2
                            nc.tensor.matmul(
                                pv[h2][:, j0:QB], v_sb[:, kk, h, :],
                                ptile[:, h2, j0:QB],
                                start=(kk == 0), stop=(kk == nkk - 1),
                                skip_group_check=True)

                    pending = None  # (kk, ptile) whose PV is not yet emitted
                    for kk in range(nkk):
                        dl = kk - 4 * tb
                        j0 = 128 * dl if dl >= 0 else 0
                        st = stp.tile([128, 2, QB], F32, tag="st", name="st")
                        for h2 in range(2):
                            p0, p1 = 64 * h2, 64 * h2 + 64
                            nc.tensor.matmul(
                                st[:, h2, j0:QB],
                                kt_sb[p0:p1, fb, kk * 128:(kk + 1) * 128],
                                qt_sb[p0:p1, fb, tb * QB + j0:(tb + 1) * QB],
                                start=True, stop=True,
                                skip_group_check=True)
                        ptile = ptp.tile([128, 2, QB], F16, tag="pt",
                                         name="ptile")
                        nc.scalar.activation(
                            ptile[:, :, j0:QB], st[:, :, j0:QB], AF.Exp)
                        if dl >= 0:
                            # zero the causally-dead triangle of the
                            # diagonal band (both heads in one strided op;
                            # the 0-step middle dim re-reads the mask tile)
                            band = ptile[:, :, j0:j0 + 128]
                            nc.vector.tensor_mul(
                                band, band,
                                bass.AP(tri.tensor, tri.offset,
                                        [tri.ap[0], [0, 2], tri.ap[1]]))
                        # PV for kk-1 goes out now: a full iteration of
                        # tensor work sits between exp(kk-1) and its PV,
                        # so exp jitter never stalls the PE queue
                        if pending is not None:
                            emit_pv(*pending)
                        pending = (kk, ptile)
                        it += 1
                        want = len(fills) * it // total_iters
                        while popped < want:
                            fills[popped]()
                            popped += 1
                        lazy_acc += lazy_rate
                        while lazy_popped < min(int(lazy_acc), len(lazy)):
                            lazy[lazy_popped]()
                            lazy_popped += 1
                    emit_pv(*pending)

                    for h2 in range(2):
                        p0, p1 = 64 * h2, 64 * h2 + 64
                        # evacuate the unnormalized y and the denominator
                        # fast so the pv bank frees for fb+1
                        ye = yep.tile([64, QB], F16, tag="ye", name="ye")
                        nc.vector.tensor_copy(ye, pv[h2][0:D, :])
                        dcp = rc1p.tile([1, QB], F32, tag="rc1", name="dcp")
                        nc.vector.tensor_copy(dcp, pv[h2][D:D + 1, :])
                        rcp1 = rc1p.tile([1, QB], F32, tag="rc1b",
                                         name="rcp1")
                        nc.vector.reciprocal_approx_fast(
                            out=rcp1, in_=dcp)
                        rcpb = rcbp.tile([64, QB], F32, tag="rcb",
                                         name="rcb")
                        nc.gpsimd.partition_broadcast(rcpb, rcp1)
                        nc.vector.tensor_mul(yt_sb[p0:p1, tb, fb, :],
                                             ye, rcpb)
                while popped < len(fills):
                    fills[popped]()
                    popped += 1
                if not SEQUENTIAL and tb < NQB - 1:
                    lazy += proj_fills(tb)

            while lazy_popped < len(lazy):
                lazy[lazy_popped]()
                lazy_popped += 1
            if SEQUENTIAL:
                for tb in range(NQB - 1):
                    for tt in range(4):
                        emit_proj_group(tb, tt)
            for tt in range(4):
                emit_proj_group(NQB - 1, tt)

    nc.compile()
    return nc


def _host_inputs(x, Wk, Wq, Wv, Wp):
    """Build the 8 per-core input maps (host-side slicing/transposes)."""
    ident_np = np.eye(128, dtype=np.float16)
    p = np.arange(128)[:, None]
    jj = np.arange(128)[None, :]
    tri_np = np.where(jj < p, 0.0, 1.0).astype(np.float16)

    in_maps = []
    for c in range(N_CORES):
        b, hg = c // 2, c % 2
        fs = slice(hg * FPC, (hg + 1) * FPC)
        in_maps.append({
            "xT": np.ascontiguousarray(x[b].T).astype(np.float16),
            "wqT": np.ascontiguousarray((Wq[fs, :] * 0.125).T).astype(np.float16),
            "wkT": np.ascontiguousarray(Wk[fs, :].T).astype(np.float16),
            "wvT": np.ascontiguousarray(Wv[fs, :].T).astype(np.float16),
            "wpT": np.ascontiguousarray(Wp[:, fs].T).astype(np.float16),
            "ident": ident_np,
            "tri": tri_np,
        })
    return in_maps


def kernel(x, Wk, Wq, Wv, Wp, bp, _trace=False):
    x = np.asarray(x, dtype=np.float32)
    Wk = np.asarray(Wk, dtype=np.float32)
    Wq = np.asarray(Wq, dtype=np.float32)
    Wv = np.asarray(Wv, dtype=np.float32)
    Wp = np.asarray(Wp, dtype=np.float32)
    bp = np.asarray(bp, dtype=np.float32)

    if "nc" not in _cached:
        _cached["nc"] = _build_program()
    nc = _cached["nc"]

    in_maps = _host_inputs(x, Wk, Wq, Wv, Wp)
    res = run_bass_kernel_spmd(nc, in_maps, core_ids=list(range(N_CORES)),
                               trace=_trace)
    _cached["last_result"] = res

    out = np.empty((B, T, C), dtype=np.float32)
    for b in range(B):
        out[b] = (res.results[2 * b]["out"].astype(np.float32)
                  + res.results[2 * b + 1]["out"]
                  + bp[None, :])
    return out
